# revision 1
# baseline (speedup 1.0000x reference)
"""2-layer multi-head GAT on 8 Trainium2 NeuronCores.

Sharding: nodes partitioned across 8 cores by dst ownership (6250 nodes each,
padded to 6272 = 49x128). Edges live on their dst's core, sorted by dst into
128-dst blocks. Per layer:
  1. per-core GEMM  feat|el|er = h @ [W | W@Al | W@Ar]   (fp32, PE)
  2. two AllGathers publish every core's projected rows (split in row-halves
     A/B so gather indices fit int16 and the second AG overlaps edge compute)
  3. per dst-block: dma_gather of src rows (1280B) + er rows (256B),
     attention e-chain (DVE/ACT), selection-matrix aggregation matmuls
     accumulated in PSUM (float32r, exact 0/1 lhsT)
  4. flush: divide by softmax denominators, ELU, transpose for next GEMM
"""
import sys
sys.path.insert(0, '/opt/trn_rl_repo')
import numpy as np

N_NODES = 50000
N_EDGES = 800000
IN_DIM = 256
HID = 64
HEADS = 4
NEG_SLOPE = 0.2
N_CORES = 8
NPC = N_NODES // N_CORES          # 6250 real nodes per core
P = 128
NB = 49                            # blocks per core
NPAD = NB * P                      # 6272 padded nodes per core
A_ROWS = 25 * P                    # 3200: local rows in table A
B_ROWS = 24 * P                    # 3072: local rows in table B
RA = N_CORES * A_ROWS              # 25600
RB = N_CORES * B_ROWS              # 24576
ES = 320                           # table row stride/elem (fp32), 1280B
CG = 260                           # feat + el columns
PAD_LDST = 999.0


def _wrap_idx(idx_list):
    """[n] int -> [128, n//16] int16 wrapped-in-16 layout, replicated."""
    n = len(idx_list)
    assert n % 16 == 0
    arr = np.asarray(idx_list, np.int16).reshape(n // 16, 16)  # [s, q]
    w16 = arr.T                                                # [16, s]
    return np.tile(w16, (8, 1))                                # [128, s]


def _prep(x, src, dst, W1, al1, ar1, W2, al2, ar2, kdt=32):
    idt = np.float16 if kdt == 16 else np.float32
    src = np.asarray(src).astype(np.int64)
    dst = np.asarray(dst).astype(np.int64)
    x = np.asarray(x, np.float32)

    # table row id for a global node n
    own = src // NPC
    loc = src % NPC
    in_a = loc < A_ROWS
    rowA = own * A_ROWS + loc                 # valid where in_a
    rowB = own * B_ROWS + (loc - A_ROWS)      # valid where ~in_a

    core_of = (dst // NPC).astype(np.int32)
    ld_all = (dst % NPC).astype(np.int32)
    blk_all = ld_all // P
    lin_all = ld_all % P

    # per (core, block): lists of A-edges and B-edges
    eA = [[[] for _ in range(NB)] for _ in range(N_CORES)]
    eB = [[[] for _ in range(NB)] for _ in range(N_CORES)]
    order = np.lexsort((src, dst))
    for e in order:
        c = core_of[e]
        b = blk_all[e]
        (eA if in_a[e] else eB)[c][b].append(e)

    T_A = [max(1, -(-max(len(eA[c][b]) for c in range(N_CORES)) // P)) for b in range(NB)]
    T_B = [max(1, -(-max(len(eB[c][b]) for c in range(N_CORES)) // P)) for b in range(NB)]
    # allow empty groups
    for b in range(NB):
        if all(len(eA[c][b]) == 0 for c in range(N_CORES)):
            T_A[b] = 0
        if all(len(eB[c][b]) == 0 for c in range(N_CORES)):
            T_B[b] = 0
    T = [T_A[b] + T_B[b] for b in range(NB)]
    TAtot, TBtot, Ttot = sum(T_A), sum(T_B), sum(T)

    plan = {"T_A": T_A, "T_B": T_B}

    # per-core tables
    in_maps = []
    Al1 = np.zeros((IN_DIM, HEADS), np.float64)
    Ar1 = np.zeros((IN_DIM, HEADS), np.float64)
    Al2 = np.zeros((IN_DIM, HEADS), np.float64)
    Ar2 = np.zeros((IN_DIM, HEADS), np.float64)
    for h in range(HEADS):
        Al1[h * HID:(h + 1) * HID, h] = np.asarray(al1, np.float64)[h]
        Ar1[h * HID:(h + 1) * HID, h] = np.asarray(ar1, np.float64)[h]
        Al2[h * HID:(h + 1) * HID, h] = np.asarray(al2, np.float64)[h]
        Ar2[h * HID:(h + 1) * HID, h] = np.asarray(ar2, np.float64)[h]

    def wext(W, Al, Ar):
        W = np.asarray(W, np.float64)
        m = np.concatenate([W, W @ Al, W @ Ar], axis=1)  # [256, 264]
        out = np.zeros((P, 2 * 264), np.float32)
        for g in range(2):
            out[:, g * 264:(g + 1) * 264] = m[g * P:(g + 1) * P].astype(np.float32)
        return out

    W1k = wext(W1, Al1, Ar1)
    W2k = wext(W2, Al2, Ar2)
    iota = np.tile(np.arange(P, dtype=idt), (P, 1))
    ident = np.eye(P, dtype=np.float32)

    for c in range(N_CORES):
        xl = np.zeros((NPAD, IN_DIM), np.float32)
        xl[:NPC] = x[c * NPC:(c + 1) * NPC]
        xT = np.zeros((P, 2 * NPAD), np.float32)
        for g in range(2):
            xT[:, g * NPAD:(g + 1) * NPAD] = xl[:, g * P:(g + 1) * P].T

        idxA_cols = []
        idxB_cols = []
        idxL_cols = []
        ldst_cols = np.full((P, max(Ttot, 1)), PAD_LDST, idt)
        toff = 0
        for b in range(NB):
            ea, eb = eA[c][b], eB[c][b]
            na, nb_ = T_A[b] * P, T_B[b] * P
            ia = [int(rowA[e]) for e in ea] + [0] * (na - len(ea))
            ib = [int(rowB[e]) for e in eb] + [0] * (nb_ - len(eb))
            il = ([int(ld_all[e]) for e in ea] + [0] * (na - len(ea))
                  + [int(ld_all[e]) for e in eb] + [0] * (nb_ - len(eb)))
            lv = ([float(lin_all[e]) for e in ea] + [PAD_LDST] * (na - len(ea))
                  + [float(lin_all[e]) for e in eb] + [PAD_LDST] * (nb_ - len(eb)))
            if na:
                idxA_cols.append(_wrap_idx(ia))
            if nb_:
                idxB_cols.append(_wrap_idx(ib))
            if na + nb_:
                idxL_cols.append(_wrap_idx(il))
                lvm = np.asarray(lv, idt).reshape(T[b], P).T  # [128, T]
                ldst_cols[:, toff:toff + T[b]] = lvm
            toff += T[b]

        in_maps.append({
            "xT": xT,
            "W1k": W1k, "W2k": W2k,
            "idxA": (np.concatenate(idxA_cols, axis=1) if idxA_cols
                     else np.zeros((P, 8), np.int16)),
            "idxB": (np.concatenate(idxB_cols, axis=1) if idxB_cols
                     else np.zeros((P, 8), np.int16)),
            "idxL": (np.concatenate(idxL_cols, axis=1) if idxL_cols
                     else np.zeros((P, 8), np.int16)),
            "ldstT": ldst_cols,
            "iota": iota, "ident": ident,
        })
    plan["idxA_cols"] = in_maps[0]["idxA"].shape[1]
    plan["idxB_cols"] = in_maps[0]["idxB"].shape[1]
    plan["idxL_cols"] = in_maps[0]["idxL"].shape[1]
    plan["ldst_cols"] = in_maps[0]["ldstT"].shape[1]
    plan["kdt"] = kdt
    return in_maps, plan


def _build(plan):
    import os
    KLVL = int(os.environ.get("KLVL", "5"))
    KSIM = int(os.environ.get("KSIM", "0"))
    import concourse.bass as bass
    import concourse.bacc as bacc
    import concourse.mybir as mybir
    import concourse.tile as tile

    dt = mybir.dt
    KDT = plan.get("kdt", 32)
    if KDT == 16:
        FDT = dt.float16          # table/feat dtype
        MDT = dt.float16          # matmul operand dtype for MT/W
        ESL = 384                 # table row elems (768B)
        ELC = 128                 # el col offset in fp32 view of a row
    else:
        FDT = dt.float32
        MDT = dt.float32r
        ESL = ES                  # 320 (1280B)
        ELC = 256
    T_A, T_B = plan["T_A"], plan["T_B"]
    T = [T_A[b] + T_B[b] for b in range(NB)]

    nc = bacc.Bacc("TRN2", target_bir_lowering=False, debug=False,
                   num_devices=(1 if KSIM else N_CORES))
    xT_ap = nc.dram_tensor("xT", [P, 2 * NPAD], dt.float32, kind="ExternalInput").ap()
    W1k_ap = nc.dram_tensor("W1k", [P, 2 * 264], dt.float32, kind="ExternalInput").ap()
    W2k_ap = nc.dram_tensor("W2k", [P, 2 * 264], dt.float32, kind="ExternalInput").ap()
    idxA_ap = nc.dram_tensor("idxA", [P, plan["idxA_cols"]], dt.int16, kind="ExternalInput").ap()
    idxB_ap = nc.dram_tensor("idxB", [P, plan["idxB_cols"]], dt.int16, kind="ExternalInput").ap()
    idxL_ap = nc.dram_tensor("idxL", [P, plan["idxL_cols"]], dt.int16, kind="ExternalInput").ap()
    ldstT_ap = nc.dram_tensor("ldstT", [P, plan["ldst_cols"]], FDT, kind="ExternalInput").ap()
    iota_ap = nc.dram_tensor("iota", [P, P], FDT, kind="ExternalInput").ap()
    ident_ap = nc.dram_tensor("ident", [P, P], dt.float32, kind="ExternalInput").ap()
    out_ap = nc.dram_tensor("out", [NPAD, IN_DIM], dt.float32, kind="ExternalOutput").ap()

    with tile.TileContext(nc) as tc:
        with tc.tile_pool(name="const", bufs=1) as cpool, \
             tc.tile_pool(name="gemm", bufs=3) as gpool, \
             tc.tile_pool(name="edge", bufs=2) as epool, \
             tc.tile_pool(name="flush", bufs=2) as fpool, \
             tc.tile_pool(name="psum", bufs=2, space="PSUM") as pp, \
             tc.tile_pool(name="dram", bufs=1, space="DRAM") as dram:

            iota_t = cpool.tile([P, P], FDT)
            ident_t = cpool.tile([P, P], dt.float32)
            idxA_t = cpool.tile([P, plan["idxA_cols"]], dt.int16)
            idxB_t = cpool.tile([P, plan["idxB_cols"]], dt.int16)
            idxL_t = cpool.tile([P, plan["idxL_cols"]], dt.int16)
            ldst_t = cpool.tile([P, plan["ldst_cols"]], FDT)
            w1_t = cpool.tile([P, 2 * 264], dt.float32)
            w2_t = cpool.tile([P, 2 * 264], dt.float32)
            nc.sync.dma_start(iota_t[:], iota_ap[:])
            nc.sync.dma_start(ident_t[:], ident_ap[:])
            nc.sync.dma_start(idxA_t[:], idxA_ap[:])
            nc.sync.dma_start(idxB_t[:], idxB_ap[:])
            nc.sync.dma_start(idxL_t[:], idxL_ap[:])
            nc.sync.dma_start(ldst_t[:], ldstT_ap[:])
            nc.sync.dma_start(w1_t[:], W1k_ap[:])
            nc.sync.dma_start(w2_t[:], W2k_ap[:])

            tabA_loc = dram.tile([A_ROWS, ESL], FDT)
            tabB_loc = dram.tile([B_ROWS, ESL], FDT)
            _ashared = "Local" if KSIM else "Shared"
            tabA1 = dram.tile([RA, ESL], FDT, addr_space=_ashared)
            tabB1 = dram.tile([RB, ESL], FDT, addr_space=_ashared)
            tabA2 = dram.tile([RA, ESL], FDT, addr_space=_ashared)
            tabB2 = dram.tile([RB, ESL], FDT, addr_space=_ashared)
            er_pad = dram.tile([NPAD, 64], dt.float32)
            h1T = dram.tile([P, 2 * NPAD], dt.float32)

            def gemm_block(layer, b):
                wk = w1_t if layer == 1 else w2_t
                ps = pp.tile([P, 264], dt.float32, space="PSUM", name="gemm_ps", tag="gemm_ps")
                for g in range(2):
                    hk = gpool.tile([P, P], dt.float32, name="hk", tag="hk")
                    if layer == 1:
                        nc.sync.dma_start(hk[:], xT_ap[:, g * NPAD + b * P: g * NPAD + (b + 1) * P])
                    else:
                        nc.sync.dma_start(hk[:], h1T[:, g * NPAD + b * P: g * NPAD + (b + 1) * P])
                    nc.tensor.matmul(out=ps[:], lhsT=hk[:], rhs=wk[:, g * 264:(g + 1) * 264],
                                     start=(g == 0), stop=(g == 1))
                sb = gpool.tile([P, 264], dt.float32, name="gemm_sb", tag="gemm_sb")
                nc.vector.tensor_copy(sb[:], ps[:])
                tab_loc = tabA_loc if b < 25 else tabB_loc
                r0 = b * P if b < 25 else (b - 25) * P
                if KDT == 16:
                    fb16 = gpool.tile([P, 256], dt.float16, name="gemm_f16", tag="gemm_f16")
                    nc.vector.tensor_copy(fb16[:], sb[:, 0:256])
                    nc.sync.dma_start(tab_loc[r0:r0 + P, 0:256], fb16[:])
                    nc.sync.dma_start(
                        tab_loc.bitcast(dt.float32)[r0:r0 + P, ELC:ELC + 4],
                        sb[:, 256:260])
                else:
                    nc.sync.dma_start(tab_loc[r0:r0 + P, 0:CG], sb[:, 0:CG])
                nc.sync.dma_start(er_pad[b * P:(b + 1) * P, 0:4], sb[:, 260:264])

            def edge_block(layer, b, toff, aoff, boff):
                ta, tb_, t = T_A[b], T_B[b], T[b]
                if t == 0 or KLVL < 3:
                    return
                tabA = tabA1 if layer == 1 else tabA2
                tabB = tabB1 if layer == 1 else tabB2
                G = epool.tile([P, t * ESL], FDT, name="G", tag="G")
                if ta:
                    nc.gpsimd.dma_gather(
                        out_ap=G[:, 0:ta * ESL].rearrange("p (t e) -> p t e", e=ESL),
                        in_ap=tabA[:], idxs_ap=idxA_t[:, 8 * aoff: 8 * (aoff + ta)],
                        num_idxs=ta * P, num_idxs_reg=ta * P, elem_size=ESL,
                        single_packet=False)
                if tb_:
                    nc.gpsimd.dma_gather(
                        out_ap=G[:, ta * ESL:t * ESL].rearrange("p (t e) -> p t e", e=ESL),
                        in_ap=tabB[:], idxs_ap=idxB_t[:, 8 * boff: 8 * (boff + tb_)],
                        num_idxs=tb_ * P, num_idxs_reg=tb_ * P, elem_size=ESL,
                        single_packet=False)
                ER = epool.tile([P, t * 64], dt.float32, name="ER", tag="ER")
                nc.gpsimd.dma_gather(
                    out_ap=ER[:].rearrange("p (t e) -> p t e", e=64),
                    in_ap=er_pad[:], idxs_ap=idxL_t[:, 8 * toff: 8 * (toff + t)],
                    num_idxs=t * P, num_idxs_reg=t * P, elem_size=64,
                    single_packet=False)

                if KLVL < 4:
                    return
                gel = G[:].bitcast(dt.float32).rearrange("p (t c) -> p t c", c=ESL // (1 if KDT == 32 else 2))
                er3 = ER[:].rearrange("p (t c) -> p t c", c=64)
                E = epool.tile([P, t * 4], dt.float32, name="E", tag="E")
                e3 = E[:].rearrange("p (t h) -> p t h", h=4)
                nc.vector.tensor_tensor(out=e3, in0=gel[:, :, ELC:ELC + 4],
                                        in1=er3[:, :, 0:4], op=mybir.AluOpType.add)
                L = epool.tile([P, t * 4], dt.float32, name="L", tag="L")
                nc.vector.tensor_scalar_mul(L[:], E[:], NEG_SLOPE)
                nc.vector.tensor_tensor(out=L[:], in0=E[:], in1=L[:],
                                        op=mybir.AluOpType.max)
                X = epool.tile([P, t * 4], dt.float32, name="X", tag="X")
                nc.scalar.activation(X[:], L[:], mybir.ActivationFunctionType.Exp)
                if KDT == 16:
                    XW = epool.tile([P, t * 4], dt.float16, name="XW", tag="XW")
                    nc.vector.tensor_copy(XW[:], X[:])
                else:
                    XW = X

                g3 = G[:].rearrange("p (t c) -> p t c", c=ESL)
                W = epool.tile([P, t * CG], MDT, name="W", tag="W")
                w3 = W[:].rearrange("p (t c) -> p t c", c=CG)
                nc.vector.tensor_copy(w3[:, :, 256:260],
                                      XW[:].rearrange("p (t h) -> p t h", h=4))
                w4 = w3[:, :, 0:256].rearrange("p t (h j) -> p t h j", j=64)
                gf4 = g3[:, :, 0:256].rearrange("p t (h j) -> p t h j", j=64)
                x4 = XW[:].rearrange("p (t h) -> p t h", h=4) \
                          .rearrange("p t (h o) -> p t h o", o=1) \
                          .to_broadcast([P, t, 4, 64])
                nc.vector.tensor_tensor(out=w4, in0=gf4, in1=x4, op=mybir.AluOpType.mult)

                MT = epool.tile([P, t * P], MDT, name="MT", tag="MT")
                mt3 = MT[:].rearrange("p (t c) -> p t c", c=P)
                iota3 = iota_t[:].rearrange("p (o c) -> p o c", o=1).to_broadcast([P, t, P])
                lds3 = ldst_t[:, toff:toff + t].rearrange("p (t o) -> p t o", o=1) \
                                               .to_broadcast([P, t, P])
                nc.vector.tensor_tensor(out=mt3, in0=iota3, in1=lds3,
                                        op=mybir.AluOpType.is_equal)

                agg = pp.tile([P, CG], dt.float32, space="PSUM", name="agg_ps", tag="agg_ps")
                for ti in range(t):
                    nc.tensor.matmul(out=agg[:], lhsT=MT[:, ti * P:(ti + 1) * P],
                                     rhs=W[:, ti * CG:(ti + 1) * CG],
                                     start=(ti == 0), stop=(ti == t - 1))

                # flush: divide by denom, ELU
                dmx = fpool.tile([P, 4], dt.float32, name="dmx", tag="dmx")
                nc.vector.tensor_scalar_max(dmx[:], agg[:, 256:260], 1e-30)
                rec = fpool.tile([P, 4], dt.float32, name="rec", tag="rec")
                nc.vector.reciprocal(rec[:], dmx[:])
                ob = fpool.tile([P, 256], dt.float32, name="ob", tag="ob")
                ob3 = ob[:].rearrange("p (h j) -> p h j", j=64)
                rec3 = rec[:].rearrange("p (h o) -> p h o", o=1).to_broadcast([P, 4, 64])
                nc.vector.tensor_tensor(out=ob3,
                                        in0=agg[:, 0:256].rearrange("p (h j) -> p h j", j=64),
                                        in1=rec3, op=mybir.AluOpType.mult)
                nb_t = fpool.tile([P, 256], dt.float32, name="nb", tag="nb")
                nc.vector.tensor_scalar_min(nb_t[:], ob[:], 0.0)
                en = fpool.tile([P, 256], dt.float32, name="en", tag="en")
                nc.scalar.activation(en[:], nb_t[:], mybir.ActivationFunctionType.Exp)
                pb = fpool.tile([P, 256], dt.float32, name="pb", tag="pb")
                nc.scalar.activation(pb[:], ob[:], mybir.ActivationFunctionType.Relu)
                fb = fpool.tile([P, 256], dt.float32, name="fb", tag="fb")
                nc.vector.tensor_tensor(out=fb[:], in0=en[:], in1=pb[:],
                                        op=mybir.AluOpType.add)
                nc.vector.tensor_scalar_add(fb[:], fb[:], -1.0)

                if KLVL < 5:
                    return
                if layer == 1:
                    for g in range(2):
                        trp = pp.tile([P, P], dt.float32, space="PSUM", name="tr_ps", tag="tr_ps")
                        nc.tensor.transpose(out=trp[:], in_=fb[:, g * P:(g + 1) * P],
                                            identity=ident_t[:])
                        tsb = fpool.tile([P, P], dt.float32, name="tsb", tag="tsb")
                        nc.vector.tensor_copy(tsb[:], trp[:])
                        nc.sync.dma_start(h1T[:, g * NPAD + b * P: g * NPAD + (b + 1) * P], tsb[:])
                else:
                    nc.sync.dma_start(out_ap[b * P:(b + 1) * P, :], fb[:])

            for layer in (1, 2):
                for b in range(25):
                    gemm_block(layer, b)
                if KLVL >= 2 and not KSIM:
                    nc.gpsimd.collective_compute(
                        "AllGather", mybir.AluOpType.bypass,
                        replica_groups=[list(range(N_CORES))],
                        ins=[tabA_loc.opt()],
                        outs=[(tabA1 if layer == 1 else tabA2).opt()])
                for b in range(25, NB):
                    gemm_block(layer, b)
                if KLVL >= 2 and not KSIM:
                    nc.gpsimd.collective_compute(
                        "AllGather", mybir.AluOpType.bypass,
                        replica_groups=[list(range(N_CORES))],
                        ins=[tabB_loc.opt()],
                        outs=[(tabB1 if layer == 1 else tabB2).opt()])
                toff = aoff = boff = 0
                for b in range(NB):
                    edge_block(layer, b, toff, aoff, boff)
                    toff += T[b]
                    aoff += T_A[b]
                    boff += T_B[b]
    nc.compile()
    return nc


def kernel(**inputs):
    import os
    from concourse.bass_utils import run_bass_kernel_spmd
    kdt = int(os.environ.get("KDT", "32"))
    in_maps, plan = _prep(inputs["x"], inputs["src"], inputs["dst"],
                          inputs["W1"], inputs["al1"], inputs["ar1"],
                          inputs["W2"], inputs["al2"], inputs["ar2"], kdt=kdt)
    nc = _build(plan)
    res = run_bass_kernel_spmd(nc, in_maps, core_ids=list(range(N_CORES)),
                               trace=False)
    h = np.concatenate([res.results[c]["out"][:NPC] for c in range(N_CORES)], axis=0)
    return tuple(h[:, i * HID:(i + 1) * HID] for i in range(HEADS))



# revision 13
# speedup vs baseline: 1.5459x; 1.5459x over previous
"""2-layer multi-head GAT on 8 Trainium2 NeuronCores (v2, fp16).

Sharding: nodes partitioned across 8 cores by dst ownership (6250 nodes each,
padded to 6272 = 49x128). Edges live on their dst's core, sorted by dst into
128-dst blocks, and split by src row-half (A: first 3200 local rows, B: rest)
so gather indices fit int16 and AllGathers pipeline with edge compute.

Per layer:
  1. per-core GEMM  feat|el|er = h @ [W | W@Al | W@Ar]  (fp16 PE, 1cyc/row).
     feat columns stored (j-major, h-minor) interleaved so the later
     alpha-broadcast multiply is a packed-last-dim DVE op (2x fp16 mode).
     er stays in SBUF (never round-trips DRAM).
  2. AllGather A-half after GEMM blocks 0-24, B-half after 25-48 (fp16 rows
     of 768B). Phase-A edge processing needs only table A, so AG(B) hides
     under it; layer-2 GEMM is interleaved into layer-1 phase-B flushes so
     AG2(A)/AG2(B) hide under remaining edge work. Only AG1(A) is exposed.
  3. per dst-block, per 128-edge tile: dma_gather of src rows (768B, quad-
     merged calls), selection matrix MT via per-tile tensor_scalar is_equal
     (4x DVE mode), er[dst] per edge via PE transpose of MT + tiny matmul
     (no 256B/edge er gather), e-chain e=lrelu(el+er), X=exp(e) written
     straight into W's denominator columns by the ACT engine, W=feat*X
     (packed 2x DVE), aggregation matmuls accumulated in PSUM (fp16).
  4. phase A stashes partial sums in SBUF; phase B combines, divides by the
     softmax denominator, applies ELU, transposes for the next GEMM.
"""
import sys
sys.path.insert(0, '/opt/trn_rl_repo')
import numpy as np

N_NODES = 50000
N_EDGES = 800000
IN_DIM = 256
HID = 64
HEADS = 4
NEG_SLOPE = 0.2
N_CORES = 8
NPC = N_NODES // N_CORES          # 6250 real nodes per core
P = 128
NB = 49                            # dst blocks per core
NPAD = NB * P                      # 6272 padded nodes per core
NBA = 25                           # blocks whose rows live in table A
A_ROWS = NBA * P                   # 3200 local rows in table A
B_ROWS = (NB - NBA) * P            # 3072 local rows in table B
RA = N_CORES * A_ROWS              # 25600
RB = N_CORES * B_ROWS              # 24576
ESL = 384                          # table row elems (fp16), 768B
CG = 260                           # feat + denom columns in W
QUAD = 4                           # blocks per merged gather call
PAD_LDST = 999.0

# feature interleave: standard col c = h*64+j  <->  stored col j*4+h
_PERM = np.arange(IN_DIM).reshape(HEADS, HID).T.reshape(-1)   # perm[j*4+h] = h*64+j


def _wrap_idx(idx_list):
    """[n] int -> [128, n//16] int16 wrapped-in-16 layout, replicated."""
    n = len(idx_list)
    assert n % 16 == 0
    arr = np.asarray(idx_list, np.int16).reshape(n // 16, 16)  # [s, q]
    w16 = arr.T                                                # [16, s]
    return np.tile(w16, (8, 1))                                # [128, s]


def _prep(x, src, dst, W1, al1, ar1, W2, al2, ar2, kdt=16):
    src = np.asarray(src).astype(np.int64)
    dst = np.asarray(dst).astype(np.int64)
    x = np.asarray(x, np.float32)

    own = src // NPC
    loc = src % NPC
    in_a = loc < A_ROWS
    rowA = own * A_ROWS + loc                 # valid where in_a
    rowB = own * B_ROWS + (loc - A_ROWS)      # valid where ~in_a

    core_of = (dst // NPC).astype(np.int32)
    ld_all = (dst % NPC).astype(np.int32)
    blk_all = ld_all // P
    lin_all = ld_all % P

    eA = [[[] for _ in range(NB)] for _ in range(N_CORES)]
    eB = [[[] for _ in range(NB)] for _ in range(N_CORES)]
    order = np.lexsort((src, dst))
    for e in order:
        c = core_of[e]
        b = blk_all[e]
        (eA if in_a[e] else eB)[c][b].append(e)

    T_A = [max(1, -(-max(len(eA[c][b]) for c in range(N_CORES)) // P)) for b in range(NB)]
    T_B = [max(1, -(-max(len(eB[c][b]) for c in range(N_CORES)) // P)) for b in range(NB)]
    for b in range(NB):
        if all(len(eA[c][b]) == 0 for c in range(N_CORES)):
            T_A[b] = 0
        if all(len(eB[c][b]) == 0 for c in range(N_CORES)):
            T_B[b] = 0

    plan = {"T_A": T_A, "T_B": T_B}

    # attention projection matrices (per-head block diagonal)
    def head_mat(a):
        m = np.zeros((IN_DIM, HEADS), np.float64)
        a = np.asarray(a, np.float64)
        for h in range(HEADS):
            m[h * HID:(h + 1) * HID, h] = a[h]
        return m

    def wext(W, al, ar, row_perm):
        """[256, 264] = [W(cols interleaved) | W@Al | W@Ar], rows optionally
        permuted (for layer 2 whose input features are interleaved)."""
        W = np.asarray(W, np.float64)
        m = np.concatenate([W[:, _PERM], W @ head_mat(al), W @ head_mat(ar)], axis=1)
        if row_perm is not None:
            m = m[row_perm]
        out = np.zeros((P, 2 * 264), np.float16)
        for g in range(2):
            out[:, g * 264:(g + 1) * 264] = m[g * P:(g + 1) * P].astype(np.float16)
        return out

    W1k = wext(W1, al1, ar1, None)
    W2k = wext(W2, al2, ar2, _PERM)
    iota = np.tile(np.arange(P, dtype=np.float16), (P, 1))
    ident = np.eye(P, dtype=np.float16)

    in_maps = []
    for c in range(N_CORES):
        xl = np.zeros((NPAD, IN_DIM), np.float32)
        xl[:NPC] = x[c * NPC:(c + 1) * NPC]
        xT = np.zeros((P, 2 * NPAD), np.float16)
        for g in range(2):
            xT[:, g * NPAD:(g + 1) * NPAD] = xl[:, g * P:(g + 1) * P].T.astype(np.float16)

        def build_phase(elists, rows, T):
            idx_cols = []
            ldst_cols = np.full((P, max(sum(T), 1)), PAD_LDST, np.float32)
            toff = 0
            for b in range(NB):
                el = elists[c][b]
                n = T[b] * P
                if n == 0:
                    continue
                ii = [int(rows[e]) for e in el] + [0] * (n - len(el))
                lv = ([float(lin_all[e]) for e in el]
                      + [PAD_LDST] * (n - len(el)))
                idx_cols.append(_wrap_idx(ii))
                ldst_cols[:, toff:toff + T[b]] = \
                    np.asarray(lv, np.float32).reshape(T[b], P).T
                toff += T[b]
            idx = (np.concatenate(idx_cols, axis=1) if idx_cols
                   else np.zeros((P, 8), np.int16))
            return idx, ldst_cols

        idxA, ldstA = build_phase(eA, rowA, T_A)
        idxB, ldstB = build_phase(eB, rowB, T_B)

        in_maps.append({
            "xT": xT, "W1k": W1k, "W2k": W2k,
            "idxA": idxA, "idxB": idxB,
            "ldstA": ldstA, "ldstB": ldstB,
            "iota": iota, "ident": ident,
        })

    plan["idxA_cols"] = in_maps[0]["idxA"].shape[1]
    plan["idxB_cols"] = in_maps[0]["idxB"].shape[1]
    plan["ldstA_cols"] = in_maps[0]["ldstA"].shape[1]
    plan["ldstB_cols"] = in_maps[0]["ldstB"].shape[1]
    return in_maps, plan


def _build(plan):
    import os
    KSIM = int(os.environ.get("KSIM", "0"))
    KAG = int(os.environ.get("KAG", "1"))
    KAGPOS = int(os.environ.get("KAGPOS", "2"))
    import concourse.bass as bass
    import concourse.bacc as bacc
    import concourse.mybir as mybir
    import concourse.tile as tile

    dt = mybir.dt
    F16 = dt.float16
    T_A, T_B = plan["T_A"], plan["T_B"]

    nc = bacc.Bacc("TRN2", target_bir_lowering=False, debug=False,
                   num_devices=(1 if KSIM else N_CORES))
    xT_ap = nc.dram_tensor("xT", [P, 2 * NPAD], F16, kind="ExternalInput").ap()
    W1k_ap = nc.dram_tensor("W1k", [P, 2 * 264], F16, kind="ExternalInput").ap()
    W2k_ap = nc.dram_tensor("W2k", [P, 2 * 264], F16, kind="ExternalInput").ap()
    idxA_ap = nc.dram_tensor("idxA", [P, plan["idxA_cols"]], dt.int16, kind="ExternalInput").ap()
    idxB_ap = nc.dram_tensor("idxB", [P, plan["idxB_cols"]], dt.int16, kind="ExternalInput").ap()
    ldstA_ap = nc.dram_tensor("ldstA", [P, plan["ldstA_cols"]], dt.float32, kind="ExternalInput").ap()
    ldstB_ap = nc.dram_tensor("ldstB", [P, plan["ldstB_cols"]], dt.float32, kind="ExternalInput").ap()
    iota_ap = nc.dram_tensor("iota", [P, P], F16, kind="ExternalInput").ap()
    ident_ap = nc.dram_tensor("ident", [P, P], F16, kind="ExternalInput").ap()
    out_ap = nc.dram_tensor("out", [NPAD, IN_DIM], dt.float32, kind="ExternalOutput").ap()

    AF = mybir.ActivationFunctionType
    ALU = mybir.AluOpType

    with tile.TileContext(nc) as tc:
        with tc.tile_pool(name="const", bufs=1) as cpool, \
             tc.tile_pool(name="gemm", bufs=2) as gpool, \
             tc.tile_pool(name="edge", bufs=2) as epool, \
             tc.tile_pool(name="flush", bufs=2) as fpool, \
             tc.tile_pool(name="psum", bufs=2, space="PSUM") as pp, \
             tc.tile_pool(name="dram", bufs=1, space="DRAM") as dram:

            iota_t = cpool.tile([P, P], F16)
            ident_t = cpool.tile([P, P], F16)
            idxA_t = cpool.tile([P, plan["idxA_cols"]], dt.int16)
            idxB_t = cpool.tile([P, plan["idxB_cols"]], dt.int16)
            ldstA_t = cpool.tile([P, plan["ldstA_cols"]], dt.float32)
            ldstB_t = cpool.tile([P, plan["ldstB_cols"]], dt.float32)
            w1_t = cpool.tile([P, 2 * 264], F16)
            w2_t = cpool.tile([P, 2 * 264], F16)
            nc.sync.dma_start(iota_t[:], iota_ap[:])
            nc.sync.dma_start(ident_t[:], ident_ap[:])
            nc.sync.dma_start(idxA_t[:], idxA_ap[:])
            nc.sync.dma_start(idxB_t[:], idxB_ap[:])
            nc.sync.dma_start(ldstA_t[:], ldstA_ap[:])
            nc.sync.dma_start(ldstB_t[:], ldstB_ap[:])
            nc.sync.dma_start(w1_t[:], W1k_ap[:])
            nc.sync.dma_start(w2_t[:], W2k_ap[:])

            # per-layer er values [dst-lane, 4], SBUF resident
            er_all = [cpool.tile([P, NB * HEADS], F16, name=f"er_all{i}")
                      for i in range(2)]
            # phase-A partial aggregation stash
            stash = cpool.tile([P, NB * CG], F16)

            tabA_loc = dram.tile([A_ROWS, ESL], F16)
            tabB_loc = dram.tile([B_ROWS, ESL], F16)
            _ashared = "Local" if KSIM else "Shared"
            tabA1 = dram.tile([RA, ESL], F16, addr_space=_ashared)
            tabB1 = dram.tile([RB, ESL], F16, addr_space=_ashared)
            tabA2 = dram.tile([RA, ESL], F16, addr_space=_ashared)
            tabB2 = dram.tile([RB, ESL], F16, addr_space=_ashared)
            h1T = dram.tile([P, 2 * NPAD], F16)

            def gemm_block(layer, b):
                wk = w1_t if layer == 1 else w2_t
                ps = pp.tile([P, 264], dt.float32, space="PSUM", name="gemm_ps", tag="gemm_ps")
                for g in range(2):
                    hk = gpool.tile([P, P], F16, name="hk", tag="hk")
                    if layer == 1:
                        nc.sync.dma_start(hk[:], xT_ap[:, g * NPAD + b * P: g * NPAD + (b + 1) * P])
                    else:
                        nc.sync.dma_start(hk[:], h1T[:, g * NPAD + b * P: g * NPAD + (b + 1) * P])
                    nc.tensor.matmul(out=ps[:], lhsT=hk[:], rhs=wk[:, g * 264:(g + 1) * 264],
                                     start=(g == 0), stop=(g == 1))
                sb = gpool.tile([P, ESL], F16, name="gemm_sb", tag="gemm_sb")
                nc.scalar.activation(sb[:, 0:CG], ps[:, 0:CG], AF.Copy)
                nc.vector.memset(sb[:, CG:ESL], 0.0)
                nc.vector.tensor_copy(er_all[layer - 1][:, b * 4:(b + 1) * 4], ps[:, 260:264])
                tab_loc = tabA_loc if b < NBA else tabB_loc
                r0 = b * P if b < NBA else (b - NBA) * P
                nc.sync.dma_start(tab_loc[r0:r0 + P, :], sb[:])

            def edge_phase(layer, phase):
                T = T_A if phase == 0 else T_B
                idx_t = idxA_t if phase == 0 else idxB_t
                ldst_t = ldstA_t if phase == 0 else ldstB_t
                if phase == 0:
                    tab = tabA1 if layer == 1 else tabA2
                else:
                    tab = tabB1 if layer == 1 else tabB2
                erl = er_all[layer - 1]

                # quad-merged gathers
                toff = 0  # tile offset within this phase
                for q0 in range(0, NB, QUAD):
                    blocks = [b for b in range(q0, min(q0 + QUAD, NB)) if T[b] > 0]
                    tq = sum(T[b] for b in blocks)
                    if tq == 0:
                        for b in range(q0, min(q0 + QUAD, NB)):
                            finish_block(layer, phase, b, None, None)
                        continue
                    G = epool.tile([P, tq * ESL], F16, name="G", tag="G")
                    nc.gpsimd.dma_gather(
                        out_ap=G[:].rearrange("p (t e) -> p t e", e=ESL),
                        in_ap=tab[:], idxs_ap=idx_t[:, 8 * toff: 8 * (toff + tq)],
                        num_idxs=tq * P, num_idxs_reg=tq * P, elem_size=ESL,
                        single_packet=False)
                    goff = 0  # tile offset within G
                    for b in range(q0, min(q0 + QUAD, NB)):
                        t = T[b]
                        if t == 0:
                            finish_block(layer, phase, b, None, None)
                            continue
                        process_block(layer, phase, b, t, G, goff, ldst_t, toff + goff, erl)
                        goff += t
                    toff += tq

            def process_block(layer, phase, b, t, G, goff, ldst_t, loff, erl):
                # selection matrix MT[e, d] = (d == ldst[e])
                MT = epool.tile([P, t * P], F16, name="MT", tag="MT")
                for ti in range(t):
                    nc.vector.tensor_scalar(
                        out=MT[:, ti * P:(ti + 1) * P], in0=iota_t[:],
                        scalar1=ldst_t[:, loff + ti: loff + ti + 1], scalar2=None,
                        op0=ALU.is_equal)
                # MT2 = MT^T per tile (PE transpose, 4 tiles per PSUM bank)
                MT2 = epool.tile([P, t * P], F16, name="MT2", tag="MT2")
                for t0 in range(0, t, 4):
                    n4 = min(4, t - t0)
                    trp = pp.tile([P, 4 * P], F16, space="PSUM", name="tr_ps", tag="tr_ps")
                    for k in range(n4):
                        nc.tensor.transpose(out=trp[:, k * P:(k + 1) * P],
                                            in_=MT[:, (t0 + k) * P:(t0 + k + 1) * P],
                                            identity=ident_t[:])
                    nc.scalar.activation(MT2[:, t0 * P:(t0 + n4) * P],
                                         trp[:, 0:n4 * P], AF.Copy)
                # er per edge: ER[e, h] = sum_c MT2[c, e] * er[c, h]
                er_ps = pp.tile([P, t * 4], dt.float32, space="PSUM", name="er_ps", tag="er_ps", bufs=1)
                for ti in range(t):
                    nc.tensor.matmul(out=er_ps[:, ti * 4:(ti + 1) * 4],
                                     lhsT=MT2[:, ti * P:(ti + 1) * P],
                                     rhs=erl[:, b * 4:(b + 1) * 4],
                                     start=True, stop=True, skip_group_check=True)
                # e-chain
                g3 = G[:, goff * ESL:(goff + t) * ESL].rearrange("p (t c) -> p t c", c=ESL)
                E = epool.tile([P, t * 4], dt.float32, name="E", tag="E")
                e3 = E[:].rearrange("p (t h) -> p t h", h=4)
                nc.vector.tensor_tensor(out=e3, in0=g3[:, :, 256:260],
                                        in1=er_ps[:].rearrange("p (t h) -> p t h", h=4),
                                        op=ALU.add)
                L = epool.tile([P, t * 4], dt.float32, name="L", tag="L")
                nc.vector.tensor_scalar_mul(L[:], E[:], NEG_SLOPE)
                nc.vector.tensor_tensor(out=L[:], in0=E[:], in1=L[:], op=ALU.max)
                # W = [feat * X | X], X written straight into cols 256:260 by ACT
                W = epool.tile([P, t * CG], F16, name="W", tag="W")
                w3 = W[:].rearrange("p (t c) -> p t c", c=CG)
                nc.scalar.activation(w3[:, :, 256:260],
                                     L[:].rearrange("p (t h) -> p t h", h=4), AF.Exp)
                w4 = w3[:, :, 0:256].rearrange("p t (j h) -> p t j h", h=4)
                gf4 = g3[:, :, 0:256].rearrange("p t (j h) -> p t j h", h=4)
                x4 = w3[:, :, 256:260].rearrange("p t (o h) -> p t o h", o=1) \
                                      .to_broadcast([P, t, 64, 4])
                nc.vector.tensor_tensor(out=w4, in0=gf4, in1=x4, op=ALU.mult)
                # aggregate
                agg = pp.tile([P, CG], dt.float32, space="PSUM", name="agg_ps", tag="agg_ps")
                for ti in range(t):
                    nc.tensor.matmul(out=agg[:], lhsT=MT[:, ti * P:(ti + 1) * P],
                                     rhs=W[:, ti * CG:(ti + 1) * CG],
                                     start=(ti == 0), stop=(ti == t - 1))
                finish_block(layer, phase, b, agg, None)

            def finish_block(layer, phase, b, agg, _unused):
                if phase == 0:
                    # stash phase-A partials (or zeros if no A edges)
                    if agg is None:
                        nc.vector.memset(stash[:, b * CG:(b + 1) * CG], 0.0)
                    else:
                        nc.scalar.activation(stash[:, b * CG:(b + 1) * CG],
                                             agg[:], AF.Copy)
                    return
                # phase B: combine + softmax divide + ELU
                comb = fpool.tile([P, CG], dt.float32, name="comb", tag="comb")
                if agg is None:
                    nc.vector.tensor_copy(comb[:], stash[:, b * CG:(b + 1) * CG])
                else:
                    nc.vector.tensor_tensor(out=comb[:], in0=stash[:, b * CG:(b + 1) * CG],
                                            in1=agg[:], op=ALU.add)
                dmx = fpool.tile([P, 4], dt.float32, name="dmx", tag="dmx")
                nc.vector.tensor_scalar_max(dmx[:], comb[:, 256:260], 1e-30)
                rec = fpool.tile([P, 4], dt.float32, name="rec", tag="rec")
                nc.vector.reciprocal(rec[:], dmx[:])
                ob = fpool.tile([P, 256], dt.float32, name="ob", tag="ob")
                ob4 = ob[:].rearrange("p (j h) -> p j h", h=4)
                rec4 = rec[:].rearrange("p (o h) -> p o h", o=1).to_broadcast([P, 64, 4])
                nc.vector.tensor_tensor(out=ob4,
                                        in0=comb[:, 0:256].rearrange("p (j h) -> p j h", h=4),
                                        in1=rec4, op=ALU.mult)
                # ELU: relu(x) + exp(min(x,0)) - 1
                nb_t = fpool.tile([P, 256], dt.float32, name="nb", tag="nb")
                nc.vector.tensor_scalar_min(nb_t[:], ob[:], 0.0)
                en = fpool.tile([P, 256], dt.float32, name="en", tag="en")
                nc.scalar.activation(en[:], nb_t[:], AF.Exp)
                pb = fpool.tile([P, 256], dt.float32, name="pb", tag="pb")
                nc.scalar.activation(pb[:], ob[:], AF.Relu)
                if layer == 1:
                    fb = fpool.tile([P, 256], F16, name="fb", tag="fb")
                    nc.vector.tensor_tensor(out=fb[:], in0=en[:], in1=pb[:], op=ALU.add)
                    nc.vector.tensor_scalar_add(fb[:], fb[:], -1.0)
                    for g in range(2):
                        trp = pp.tile([P, P], F16, space="PSUM", name="tr2_ps", tag="tr2_ps", bufs=1)
                        nc.tensor.transpose(out=trp[:], in_=fb[:, g * P:(g + 1) * P],
                                            identity=ident_t[:])
                        tsb = fpool.tile([P, P], F16, name="tsb", tag="tsb")
                        nc.scalar.activation(tsb[:], trp[:], AF.Copy)
                        nc.sync.dma_start(h1T[:, g * NPAD + b * P: g * NPAD + (b + 1) * P], tsb[:])
                    # layer-2 GEMM for this block as soon as its h1 lands
                    gemm_block(2, b)
                    if KAGPOS:
                        agb = NBA - 1 if KAGPOS == 1 else (NB - 2 if KAGPOS == 2 else NB - 1)
                        if b == agb:
                            ag(tabA_loc, tabA2)
                        elif b == NB - 1:
                            ag(tabB_loc, tabB2)
                else:
                    fb = fpool.tile([P, 256], dt.float32, name="fb32", tag="fb32")
                    nc.vector.tensor_tensor(out=fb[:], in0=en[:], in1=pb[:], op=ALU.add)
                    nc.vector.tensor_scalar_add(fb[:], fb[:], -1.0)
                    nc.sync.dma_start(out_ap[b * P:(b + 1) * P, :], fb[:])

            def ag(src_tile, dst_tile):
                if KSIM or not KAG:
                    return
                nc.gpsimd.collective_compute(
                    "AllGather", mybir.AluOpType.bypass,
                    replica_groups=[list(range(N_CORES))],
                    ins=[src_tile.opt()],
                    outs=[dst_tile.opt()])

            # ---- schedule ----
            for b in range(NBA):
                gemm_block(1, b)
            ag(tabA_loc, tabA1)
            for b in range(NBA, NB):
                gemm_block(1, b)
            ag(tabB_loc, tabB1)
            edge_phase(1, 0)
            edge_phase(1, 1)   # interleaves gemm_block(2, b) + AG2 launches
            if not KAGPOS:
                ag(tabA_loc, tabA2)
                ag(tabB_loc, tabB2)
            edge_phase(2, 0)
            edge_phase(2, 1)

    nc.compile()
    return nc


def _finish(results):
    """Per-core 'out' [NPAD, 256] (cols interleaved j*4+h) -> tuple of heads."""
    h = np.concatenate([np.asarray(results[c]["out"])[:NPC] for c in range(N_CORES)],
                       axis=0)
    h = h.reshape(N_NODES, HID, HEADS).transpose(0, 2, 1)   # [N, H, D]
    return tuple(h[:, i] for i in range(HEADS))


def kernel(**inputs):
    from concourse.bass_utils import run_bass_kernel_spmd
    in_maps, plan = _prep(inputs["x"], inputs["src"], inputs["dst"],
                          inputs["W1"], inputs["al1"], inputs["ar1"],
                          inputs["W2"], inputs["al2"], inputs["ar2"])
    nc = _build(plan)
    res = run_bass_kernel_spmd(nc, in_maps, core_ids=list(range(N_CORES)),
                               trace=False)
    return _finish(res.results)


# revision 17
# speedup vs baseline: 1.6946x; 1.0962x over previous
"""2-layer multi-head GAT on 8 Trainium2 NeuronCores (v2, fp16).

Sharding: nodes partitioned across 8 cores by dst ownership (6250 nodes each,
padded to 6272 = 49x128). Edges live on their dst's core, sorted by dst into
128-dst blocks, and split by src row-half (A: first 3200 local rows, B: rest)
so gather indices fit int16 and AllGathers pipeline with edge compute.

Per layer:
  1. per-core GEMM  feat|el|er = h @ [W | W@Al | W@Ar]  (fp16 PE, 1cyc/row).
     feat columns stored (j-major, h-minor) interleaved so the later
     alpha-broadcast multiply is a packed-last-dim DVE op (2x fp16 mode).
     er stays in SBUF (never round-trips DRAM).
  2. AllGather A-half after GEMM blocks 0-24, B-half after 25-48 (fp16 rows
     of 768B). Phase-A edge processing needs only table A, so AG(B) hides
     under it; layer-2 GEMM is interleaved into layer-1 phase-B flushes so
     AG2(A)/AG2(B) hide under remaining edge work. Only AG1(A) is exposed.
  3. per dst-block, per 128-edge tile: dma_gather of src rows (768B, quad-
     merged calls), selection matrix MT via per-tile tensor_scalar is_equal
     (4x DVE mode), er[dst] per edge via PE transpose of MT + tiny matmul
     (no 256B/edge er gather), e-chain e=lrelu(el+er), X=exp(e) written
     straight into W's denominator columns by the ACT engine, W=feat*X
     (packed 2x DVE), aggregation matmuls accumulated in PSUM (fp16).
  4. phase A stashes partial sums in SBUF; phase B combines, divides by the
     softmax denominator, applies ELU, transposes for the next GEMM.
"""
import sys
sys.path.insert(0, '/opt/trn_rl_repo')
import numpy as np

N_NODES = 50000
N_EDGES = 800000
IN_DIM = 256
HID = 64
HEADS = 4
NEG_SLOPE = 0.2
N_CORES = 8
NPC = N_NODES // N_CORES          # 6250 real nodes per core
P = 128
NB = 49                            # dst blocks per core
NPAD = NB * P                      # 6272 padded nodes per core
NBA = 25                           # blocks whose rows live in table A
A_ROWS = NBA * P                   # 3200 local rows in table A
B_ROWS = (NB - NBA) * P            # 3072 local rows in table B
RA = N_CORES * A_ROWS              # 25600
RB = N_CORES * B_ROWS              # 24576
ESL = 384                          # table row elems (fp16), 768B
CG = 260                           # feat + denom columns in W
QUAD = 4                           # blocks per merged gather call
PAD_LDST = 999.0

# feature interleave: standard col c = h*64+j  <->  stored col j*4+h
_PERM = np.arange(IN_DIM).reshape(HEADS, HID).T.reshape(-1)   # perm[j*4+h] = h*64+j


def _wrap_idx(idx_list):
    """[n] int -> [128, n//16] int16 wrapped-in-16 layout, replicated."""
    n = len(idx_list)
    assert n % 16 == 0
    arr = np.asarray(idx_list, np.int16).reshape(n // 16, 16)  # [s, q]
    w16 = arr.T                                                # [16, s]
    return np.tile(w16, (8, 1))                                # [128, s]


def _prep(x, src, dst, W1, al1, ar1, W2, al2, ar2, kdt=16):
    src = np.asarray(src).astype(np.int64)
    dst = np.asarray(dst).astype(np.int64)
    x = np.asarray(x, np.float32)

    own = src // NPC
    loc = src % NPC
    in_a = loc < A_ROWS
    rowA = own * A_ROWS + loc                 # valid where in_a
    rowB = own * B_ROWS + (loc - A_ROWS)      # valid where ~in_a

    core_of = (dst // NPC).astype(np.int32)
    ld_all = (dst % NPC).astype(np.int32)
    blk_all = ld_all // P
    lin_all = ld_all % P

    eA = [[[] for _ in range(NB)] for _ in range(N_CORES)]
    eB = [[[] for _ in range(NB)] for _ in range(N_CORES)]
    order = np.lexsort((src, dst))
    for e in order:
        c = core_of[e]
        b = blk_all[e]
        (eA if in_a[e] else eB)[c][b].append(e)

    T_A = [max(1, -(-max(len(eA[c][b]) for c in range(N_CORES)) // P)) for b in range(NB)]
    T_B = [max(1, -(-max(len(eB[c][b]) for c in range(N_CORES)) // P)) for b in range(NB)]
    for b in range(NB):
        if all(len(eA[c][b]) == 0 for c in range(N_CORES)):
            T_A[b] = 0
        if all(len(eB[c][b]) == 0 for c in range(N_CORES)):
            T_B[b] = 0

    plan = {"T_A": T_A, "T_B": T_B}

    # attention projection matrices (per-head block diagonal)
    def head_mat(a):
        m = np.zeros((IN_DIM, HEADS), np.float64)
        a = np.asarray(a, np.float64)
        for h in range(HEADS):
            m[h * HID:(h + 1) * HID, h] = a[h]
        return m

    def wext(W, al, ar, row_perm):
        """[256, 264] = [W(cols interleaved) | W@Al | W@Ar], rows optionally
        permuted (for layer 2 whose input features are interleaved)."""
        W = np.asarray(W, np.float64)
        m = np.concatenate([W[:, _PERM], W @ head_mat(al), W @ head_mat(ar)], axis=1)
        if row_perm is not None:
            m = m[row_perm]
        out = np.zeros((P, 2 * 264), np.float16)
        for g in range(2):
            out[:, g * 264:(g + 1) * 264] = m[g * P:(g + 1) * P].astype(np.float16)
        return out

    W1k = wext(W1, al1, ar1, None)
    W2k = wext(W2, al2, ar2, _PERM)
    iota = np.tile(np.arange(P, dtype=np.float16), (P, 1))
    ident = np.eye(P, dtype=np.float16)

    in_maps = []
    for c in range(N_CORES):
        xl = np.zeros((NPAD, IN_DIM), np.float32)
        xl[:NPC] = x[c * NPC:(c + 1) * NPC]
        xT = np.zeros((P, 2 * NPAD), np.float16)
        for g in range(2):
            xT[:, g * NPAD:(g + 1) * NPAD] = xl[:, g * P:(g + 1) * P].T.astype(np.float16)

        def build_phase(elists, rows, T):
            idx_cols = []
            ldst_cols = np.full((P, max(sum(T), 1)), PAD_LDST, np.float32)
            toff = 0
            for b in range(NB):
                el = elists[c][b]
                n = T[b] * P
                if n == 0:
                    continue
                ii = [int(rows[e]) for e in el] + [0] * (n - len(el))
                lv = ([float(lin_all[e]) for e in el]
                      + [PAD_LDST] * (n - len(el)))
                idx_cols.append(_wrap_idx(ii))
                ldst_cols[:, toff:toff + T[b]] = \
                    np.asarray(lv, np.float32).reshape(T[b], P).T
                toff += T[b]
            idx = (np.concatenate(idx_cols, axis=1) if idx_cols
                   else np.zeros((P, 8), np.int16))
            return idx, ldst_cols

        idxA, ldstA = build_phase(eA, rowA, T_A)
        idxB, ldstB = build_phase(eB, rowB, T_B)

        in_maps.append({
            "xT": xT, "W1k": W1k, "W2k": W2k,
            "idxA": idxA, "idxB": idxB,
            "ldstA": ldstA, "ldstB": ldstB,
            "iota": iota, "ident": ident,
        })

    plan["idxA_cols"] = in_maps[0]["idxA"].shape[1]
    plan["idxB_cols"] = in_maps[0]["idxB"].shape[1]
    plan["ldstA_cols"] = in_maps[0]["ldstA"].shape[1]
    plan["ldstB_cols"] = in_maps[0]["ldstB"].shape[1]
    return in_maps, plan


def _build(plan):
    import os
    KSIM = int(os.environ.get("KSIM", "0"))
    KAG = int(os.environ.get("KAG", "1"))
    KAGPOS = int(os.environ.get("KAGPOS", "2"))
    import concourse.bass as bass
    import concourse.bacc as bacc
    import concourse.mybir as mybir
    import concourse.tile as tile

    dt = mybir.dt
    F16 = dt.float16
    T_A, T_B = plan["T_A"], plan["T_B"]

    nc = bacc.Bacc("TRN2", target_bir_lowering=False, debug=False,
                   num_devices=(1 if KSIM else N_CORES),
                   num_swdge_queues=4)
    xT_ap = nc.dram_tensor("xT", [P, 2 * NPAD], F16, kind="ExternalInput").ap()
    W1k_ap = nc.dram_tensor("W1k", [P, 2 * 264], F16, kind="ExternalInput").ap()
    W2k_ap = nc.dram_tensor("W2k", [P, 2 * 264], F16, kind="ExternalInput").ap()
    idxA_ap = nc.dram_tensor("idxA", [P, plan["idxA_cols"]], dt.int16, kind="ExternalInput").ap()
    idxB_ap = nc.dram_tensor("idxB", [P, plan["idxB_cols"]], dt.int16, kind="ExternalInput").ap()
    ldstA_ap = nc.dram_tensor("ldstA", [P, plan["ldstA_cols"]], dt.float32, kind="ExternalInput").ap()
    ldstB_ap = nc.dram_tensor("ldstB", [P, plan["ldstB_cols"]], dt.float32, kind="ExternalInput").ap()
    iota_ap = nc.dram_tensor("iota", [P, P], F16, kind="ExternalInput").ap()
    ident_ap = nc.dram_tensor("ident", [P, P], F16, kind="ExternalInput").ap()
    out_ap = nc.dram_tensor("out", [NPAD, IN_DIM], dt.float32, kind="ExternalOutput").ap()

    AF = mybir.ActivationFunctionType
    ALU = mybir.AluOpType

    with tile.TileContext(nc) as tc:
        with tc.tile_pool(name="const", bufs=1) as cpool, \
             tc.tile_pool(name="gemm", bufs=2) as gpool, \
             tc.tile_pool(name="edge", bufs=2) as epool, \
             tc.tile_pool(name="flush", bufs=2) as fpool, \
             tc.tile_pool(name="psum", bufs=2, space="PSUM") as pp, \
             tc.tile_pool(name="dram", bufs=1, space="DRAM") as dram:

            iota_t = cpool.tile([P, P], F16)
            ident_t = cpool.tile([P, P], F16)
            idxA_t = cpool.tile([P, plan["idxA_cols"]], dt.int16)
            idxB_t = cpool.tile([P, plan["idxB_cols"]], dt.int16)
            ldstA_t = cpool.tile([P, plan["ldstA_cols"]], dt.float32)
            ldstB_t = cpool.tile([P, plan["ldstB_cols"]], dt.float32)
            w1_t = cpool.tile([P, 2 * 264], F16)
            w2_t = cpool.tile([P, 2 * 264], F16)
            nc.sync.dma_start(iota_t[:], iota_ap[:])
            nc.sync.dma_start(ident_t[:], ident_ap[:])
            nc.sync.dma_start(idxA_t[:], idxA_ap[:])
            nc.sync.dma_start(idxB_t[:], idxB_ap[:])
            nc.sync.dma_start(ldstA_t[:], ldstA_ap[:])
            nc.sync.dma_start(ldstB_t[:], ldstB_ap[:])
            nc.sync.dma_start(w1_t[:], W1k_ap[:])
            nc.sync.dma_start(w2_t[:], W2k_ap[:])

            # per-layer er values [dst-lane, 4], SBUF resident
            er_all = [cpool.tile([P, NB * HEADS], F16, name=f"er_all{i}")
                      for i in range(2)]
            # phase-A partial aggregation stash
            stash = cpool.tile([P, NB * CG], F16)

            tabA_loc = dram.tile([A_ROWS, ESL], F16)
            tabB_loc = dram.tile([B_ROWS, ESL], F16)
            _ashared = "Local" if KSIM else "Shared"
            tabA1 = dram.tile([RA, ESL], F16, addr_space=_ashared)
            tabB1 = dram.tile([RB, ESL], F16, addr_space=_ashared)
            tabA2 = dram.tile([RA, ESL], F16, addr_space=_ashared)
            tabB2 = dram.tile([RB, ESL], F16, addr_space=_ashared)
            h1T = dram.tile([P, 2 * NPAD], F16)

            def gemm_block(layer, b):
                wk = w1_t if layer == 1 else w2_t
                ps = pp.tile([P, 264], dt.float32, space="PSUM", name="gemm_ps", tag="gemm_ps")
                for g in range(2):
                    hk = gpool.tile([P, P], F16, name="hk", tag="hk")
                    if layer == 1:
                        nc.sync.dma_start(hk[:], xT_ap[:, g * NPAD + b * P: g * NPAD + (b + 1) * P])
                    else:
                        nc.sync.dma_start(hk[:], h1T[:, g * NPAD + b * P: g * NPAD + (b + 1) * P])
                    nc.tensor.matmul(out=ps[:], lhsT=hk[:], rhs=wk[:, g * 264:(g + 1) * 264],
                                     start=(g == 0), stop=(g == 1))
                sb = gpool.tile([P, ESL], F16, name="gemm_sb", tag="gemm_sb")
                nc.scalar.activation(sb[:, 0:CG], ps[:, 0:CG], AF.Copy)
                nc.vector.memset(sb[:, CG:ESL], 0.0)
                nc.vector.tensor_copy(er_all[layer - 1][:, b * 4:(b + 1) * 4], ps[:, 260:264])
                tab_loc = tabA_loc if b < NBA else tabB_loc
                r0 = b * P if b < NBA else (b - NBA) * P
                nc.sync.dma_start(tab_loc[r0:r0 + P, :], sb[:])

            def edge_phase(layer, phase):
                T = T_A if phase == 0 else T_B
                idx_t = idxA_t if phase == 0 else idxB_t
                ldst_t = ldstA_t if phase == 0 else ldstB_t
                if phase == 0:
                    tab = tabA1 if layer == 1 else tabA2
                else:
                    tab = tabB1 if layer == 1 else tabB2
                erl = er_all[layer - 1]

                # quad-merged gathers
                toff = 0  # tile offset within this phase
                for q0 in range(0, NB, QUAD):
                    blocks = [b for b in range(q0, min(q0 + QUAD, NB)) if T[b] > 0]
                    tq = sum(T[b] for b in blocks)
                    if tq == 0:
                        for b in range(q0, min(q0 + QUAD, NB)):
                            finish_block(layer, phase, b, None, None)
                        continue
                    G = epool.tile([P, tq * ESL], F16, name="G", tag="G")
                    nc.gpsimd.dma_gather(
                        out_ap=G[:].rearrange("p (t e) -> p t e", e=ESL),
                        in_ap=tab[:], idxs_ap=idx_t[:, 8 * toff: 8 * (toff + tq)],
                        num_idxs=tq * P, num_idxs_reg=tq * P, elem_size=ESL,
                        single_packet=False, queue_num=(q0 // QUAD) % 4)
                    goff = 0  # tile offset within G
                    for b in range(q0, min(q0 + QUAD, NB)):
                        t = T[b]
                        if t == 0:
                            finish_block(layer, phase, b, None, None)
                            continue
                        process_block(layer, phase, b, t, G, goff, ldst_t, toff + goff, erl)
                        goff += t
                    toff += tq

            def process_block(layer, phase, b, t, G, goff, ldst_t, loff, erl):
                # selection matrix MT[e, d] = (d == ldst[e])
                MT = epool.tile([P, t * P], F16, name="MT", tag="MT")
                for ti in range(t):
                    nc.vector.tensor_scalar(
                        out=MT[:, ti * P:(ti + 1) * P], in0=iota_t[:],
                        scalar1=ldst_t[:, loff + ti: loff + ti + 1], scalar2=None,
                        op0=ALU.is_equal)
                # MT2 = MT^T per tile (PE transpose, 8 tiles per 2KB PSUM bank)
                MT2 = epool.tile([P, t * P], F16, name="MT2", tag="MT2")
                for t0 in range(0, t, 8):
                    n8 = min(8, t - t0)
                    trp = pp.tile([P, 8 * P], F16, space="PSUM", name="tr_ps", tag="tr_ps")
                    for k in range(n8):
                        nc.tensor.transpose(out=trp[:, k * P:(k + 1) * P],
                                            in_=MT[:, (t0 + k) * P:(t0 + k + 1) * P],
                                            identity=ident_t[:])
                    nc.scalar.activation(MT2[:, t0 * P:(t0 + n8) * P],
                                         trp[:, 0:n8 * P], AF.Copy)
                # er per edge: ER[e, h] = sum_c MT2[c, e] * er[c, h]
                er_ps = pp.tile([P, t * 4], dt.float32, space="PSUM", name="er_ps", tag="er_ps", bufs=1)
                for ti in range(t):
                    nc.tensor.matmul(out=er_ps[:, ti * 4:(ti + 1) * 4],
                                     lhsT=MT2[:, ti * P:(ti + 1) * P],
                                     rhs=erl[:, b * 4:(b + 1) * 4],
                                     start=True, stop=True, skip_group_check=True)
                # e-chain
                g3 = G[:, goff * ESL:(goff + t) * ESL].rearrange("p (t c) -> p t c", c=ESL)
                E = epool.tile([P, t * 4], dt.float32, name="E", tag="E")
                e3 = E[:].rearrange("p (t h) -> p t h", h=4)
                nc.vector.tensor_tensor(out=e3, in0=g3[:, :, 256:260],
                                        in1=er_ps[:].rearrange("p (t h) -> p t h", h=4),
                                        op=ALU.add)
                L = epool.tile([P, t * 4], dt.float32, name="L", tag="L")
                nc.vector.tensor_scalar_mul(L[:], E[:], NEG_SLOPE)
                nc.vector.tensor_tensor(out=L[:], in0=E[:], in1=L[:], op=ALU.max)
                # W = [feat * X | X], X written straight into cols 256:260 by ACT
                W = epool.tile([P, t * CG], F16, name="W", tag="W")
                w3 = W[:].rearrange("p (t c) -> p t c", c=CG)
                nc.scalar.activation(w3[:, :, 256:260],
                                     L[:].rearrange("p (t h) -> p t h", h=4), AF.Exp)
                w4 = w3[:, :, 0:256].rearrange("p t (j h) -> p t j h", h=4)
                gf4 = g3[:, :, 0:256].rearrange("p t (j h) -> p t j h", h=4)
                x4 = w3[:, :, 256:260].rearrange("p t (o h) -> p t o h", o=1) \
                                      .to_broadcast([P, t, 64, 4])
                nc.vector.tensor_tensor(out=w4, in0=gf4, in1=x4, op=ALU.mult)
                # aggregate
                agg = pp.tile([P, CG], dt.float32, space="PSUM", name="agg_ps", tag="agg_ps")
                for ti in range(t):
                    nc.tensor.matmul(out=agg[:], lhsT=MT[:, ti * P:(ti + 1) * P],
                                     rhs=W[:, ti * CG:(ti + 1) * CG],
                                     start=(ti == 0), stop=(ti == t - 1))
                finish_block(layer, phase, b, agg, None)

            def finish_block(layer, phase, b, agg, _unused):
                if phase == 0:
                    # stash phase-A partials (or zeros if no A edges)
                    if agg is None:
                        nc.vector.memset(stash[:, b * CG:(b + 1) * CG], 0.0)
                    else:
                        nc.scalar.activation(stash[:, b * CG:(b + 1) * CG],
                                             agg[:], AF.Copy)
                    return
                # phase B: combine + softmax divide + ELU
                comb = fpool.tile([P, CG], dt.float32, name="comb", tag="comb")
                if agg is None:
                    nc.vector.tensor_copy(comb[:], stash[:, b * CG:(b + 1) * CG])
                else:
                    nc.vector.tensor_tensor(out=comb[:], in0=stash[:, b * CG:(b + 1) * CG],
                                            in1=agg[:], op=ALU.add)
                dmx = fpool.tile([P, 4], dt.float32, name="dmx", tag="dmx")
                nc.vector.tensor_scalar_max(dmx[:], comb[:, 256:260], 1e-30)
                rec = fpool.tile([P, 4], dt.float32, name="rec", tag="rec")
                nc.vector.reciprocal(rec[:], dmx[:])
                ob = fpool.tile([P, 256], dt.float32, name="ob", tag="ob")
                ob4 = ob[:].rearrange("p (j h) -> p j h", h=4)
                rec4 = rec[:].rearrange("p (o h) -> p o h", o=1).to_broadcast([P, 64, 4])
                nc.vector.tensor_tensor(out=ob4,
                                        in0=comb[:, 0:256].rearrange("p (j h) -> p j h", h=4),
                                        in1=rec4, op=ALU.mult)
                # ELU: relu(x) + exp(min(x,0)) - 1
                nb_t = fpool.tile([P, 256], dt.float32, name="nb", tag="nb")
                nc.vector.tensor_scalar_min(nb_t[:], ob[:], 0.0)
                en = fpool.tile([P, 256], dt.float32, name="en", tag="en")
                nc.scalar.activation(en[:], nb_t[:], AF.Exp)
                pb = fpool.tile([P, 256], dt.float32, name="pb", tag="pb")
                nc.scalar.activation(pb[:], ob[:], AF.Relu)
                if layer == 1:
                    fb = fpool.tile([P, 256], F16, name="fb", tag="fb")
                    nc.vector.tensor_tensor(out=fb[:], in0=en[:], in1=pb[:], op=ALU.add)
                    nc.vector.tensor_scalar_add(fb[:], fb[:], -1.0)
                    for g in range(2):
                        tsb = fpool.tile([P, P], F16, name="tsb", tag="tsb")
                        nc.sync.dma_start(tsb[:], fb[:, g * P:(g + 1) * P],
                                          transpose=True)
                        nc.sync.dma_start(h1T[:, g * NPAD + b * P: g * NPAD + (b + 1) * P], tsb[:])
                    # layer-2 GEMM for this block as soon as its h1 lands
                    gemm_block(2, b)
                    if KAGPOS:
                        agb = NBA - 1 if KAGPOS == 1 else (NB - 2 if KAGPOS == 2 else NB - 1)
                        if b == agb:
                            ag(tabA_loc, tabA2)
                        elif b == NB - 1:
                            ag(tabB_loc, tabB2)
                else:
                    fb = fpool.tile([P, 256], dt.float32, name="fb32", tag="fb32")
                    nc.vector.tensor_tensor(out=fb[:], in0=en[:], in1=pb[:], op=ALU.add)
                    nc.vector.tensor_scalar_add(fb[:], fb[:], -1.0)
                    nc.sync.dma_start(out_ap[b * P:(b + 1) * P, :], fb[:])

            def ag(src_tile, dst_tile):
                if KSIM or not KAG:
                    return
                nc.gpsimd.collective_compute(
                    "AllGather", mybir.AluOpType.bypass,
                    replica_groups=[list(range(N_CORES))],
                    ins=[src_tile.opt()],
                    outs=[dst_tile.opt()])

            # ---- schedule ----
            for b in range(NBA):
                gemm_block(1, b)
            ag(tabA_loc, tabA1)
            for b in range(NBA, NB):
                gemm_block(1, b)
            ag(tabB_loc, tabB1)
            edge_phase(1, 0)
            edge_phase(1, 1)   # interleaves gemm_block(2, b) + AG2 launches
            if not KAGPOS:
                ag(tabA_loc, tabA2)
                ag(tabB_loc, tabB2)
            edge_phase(2, 0)
            edge_phase(2, 1)

    nc.compile()
    return nc


def _finish(results):
    """Per-core 'out' [NPAD, 256] (cols interleaved j*4+h) -> tuple of heads."""
    h = np.concatenate([np.asarray(results[c]["out"])[:NPC] for c in range(N_CORES)],
                       axis=0)
    h = h.reshape(N_NODES, HID, HEADS).transpose(0, 2, 1)   # [N, H, D]
    return tuple(h[:, i] for i in range(HEADS))


def kernel(**inputs):
    from concourse.bass_utils import run_bass_kernel_spmd
    in_maps, plan = _prep(inputs["x"], inputs["src"], inputs["dst"],
                          inputs["W1"], inputs["al1"], inputs["ar1"],
                          inputs["W2"], inputs["al2"], inputs["ar2"])
    nc = _build(plan)
    res = run_bass_kernel_spmd(nc, in_maps, core_ids=list(range(N_CORES)),
                               trace=False)
    return _finish(res.results)


# revision 20
# speedup vs baseline: 2.1503x; 1.2689x over previous
"""2-layer multi-head GAT on 8 Trainium2 NeuronCores (v2, fp16).

Sharding: nodes partitioned across 8 cores by dst ownership (6250 nodes each,
padded to 6272 = 49x128). Edges live on their dst's core, sorted by dst into
128-dst blocks, and split by src row-half (A: first 3200 local rows, B: rest)
so gather indices fit int16 and AllGathers pipeline with edge compute.

Per layer:
  1. per-core GEMM  feat|el|er = h @ [W | W@Al | W@Ar]  (fp16 PE, 1cyc/row).
     feat columns stored (j-major, h-minor) interleaved so the later
     alpha-broadcast multiply is a packed-last-dim DVE op (2x fp16 mode).
     er stays in SBUF (never round-trips DRAM).
  2. AllGather A-half after GEMM blocks 0-24, B-half after 25-48 (fp16 rows
     of 768B). Phase-A edge processing needs only table A, so AG(B) hides
     under it; layer-2 GEMM is interleaved into layer-1 phase-B flushes so
     AG2(A)/AG2(B) hide under remaining edge work. Only AG1(A) is exposed.
  3. per dst-block, per 128-edge tile: dma_gather of src rows (768B, quad-
     merged calls), selection matrix MT via per-tile tensor_scalar is_equal
     (4x DVE mode), er[dst] per edge via PE transpose of MT + tiny matmul
     (no 256B/edge er gather), e-chain e=lrelu(el+er), X=exp(e) written
     straight into W's denominator columns by the ACT engine, W=feat*X
     (packed 2x DVE), aggregation matmuls accumulated in PSUM (fp16).
  4. phase A stashes partial sums in SBUF; phase B combines, divides by the
     softmax denominator, applies ELU, transposes for the next GEMM.
"""
import sys
sys.path.insert(0, '/opt/trn_rl_repo')
import numpy as np

N_NODES = 50000
N_EDGES = 800000
IN_DIM = 256
HID = 64
HEADS = 4
NEG_SLOPE = 0.2
N_CORES = 8
NPC = N_NODES // N_CORES          # 6250 real nodes per core
P = 128
NB = 49                            # dst blocks per core
NPAD = NB * P                      # 6272 padded nodes per core
NBA = 26                           # blocks whose rows live in table A
A_ROWS = NBA * P                   # 3200 local rows in table A
B_ROWS = (NB - NBA) * P            # 3072 local rows in table B
RA = N_CORES * A_ROWS              # 25600
RB = N_CORES * B_ROWS              # 24576
ESL = 384                          # table row elems (fp16), 768B
CG = 260                           # feat + denom columns in W
QUAD = 4                           # blocks per merged gather call
PAD_LDST = 999.0

# feature interleave: standard col c = h*64+j  <->  stored col j*4+h
_PERM = np.arange(IN_DIM).reshape(HEADS, HID).T.reshape(-1)   # perm[j*4+h] = h*64+j


def _wrap_idx(idx_list):
    """[n] int -> [128, n//16] int16 wrapped-in-16 layout, replicated."""
    n = len(idx_list)
    assert n % 16 == 0
    arr = np.asarray(idx_list, np.int16).reshape(n // 16, 16)  # [s, q]
    w16 = arr.T                                                # [16, s]
    return np.tile(w16, (8, 1))                                # [128, s]


_PLACE = {}


def _placement(dst):
    """LPT in-degree balancing: node -> (core, padded slot) so per-(core,
    block) edge counts are even across cores (less tile padding)."""
    import heapq
    deg = np.bincount(dst, minlength=N_NODES)
    order = np.argsort(-deg, kind="stable")
    NBINS = N_CORES * NB
    heap = [(0, b) for b in range(NBINS)]
    heapq.heapify(heap)
    fill = np.zeros(NBINS, np.int64)
    core_of_n = np.zeros(N_NODES, np.int32)
    slot_of_n = np.zeros(N_NODES, np.int32)
    for n in order:
        while True:
            load, b = heapq.heappop(heap)
            if fill[b] < P:
                break
        core_of_n[n] = b // NB
        slot_of_n[n] = (b % NB) * P + fill[b]
        fill[b] += 1
        heapq.heappush(heap, (load + deg[n], b))
    return core_of_n, slot_of_n


def _prep(x, src, dst, W1, al1, ar1, W2, al2, ar2, kdt=16):
    src = np.asarray(src).astype(np.int64)
    dst = np.asarray(dst).astype(np.int64)
    x = np.asarray(x, np.float32)

    core_of_n, slot_of_n = _placement(dst)
    _PLACE["core"] = core_of_n
    _PLACE["slot"] = slot_of_n

    own = core_of_n[src]
    loc = slot_of_n[src]
    in_a = loc < A_ROWS
    rowA = own.astype(np.int64) * A_ROWS + loc        # valid where in_a
    rowB = own.astype(np.int64) * B_ROWS + (loc - A_ROWS)  # valid where ~in_a

    core_of = core_of_n[dst]
    ld_all = slot_of_n[dst]
    blk_all = ld_all // P
    lin_all = ld_all % P

    eA = [[[] for _ in range(NB)] for _ in range(N_CORES)]
    eB = [[[] for _ in range(NB)] for _ in range(N_CORES)]
    order = np.lexsort((src, dst))
    for e in order:
        c = core_of[e]
        b = blk_all[e]
        (eA if in_a[e] else eB)[c][b].append(e)

    T_A = [max(1, -(-max(len(eA[c][b]) for c in range(N_CORES)) // P)) for b in range(NB)]
    T_B = [max(1, -(-max(len(eB[c][b]) for c in range(N_CORES)) // P)) for b in range(NB)]
    for b in range(NB):
        if all(len(eA[c][b]) == 0 for c in range(N_CORES)):
            T_A[b] = 0
        if all(len(eB[c][b]) == 0 for c in range(N_CORES)):
            T_B[b] = 0

    plan = {"T_A": T_A, "T_B": T_B}

    # attention projection matrices (per-head block diagonal)
    def head_mat(a):
        m = np.zeros((IN_DIM, HEADS), np.float64)
        a = np.asarray(a, np.float64)
        for h in range(HEADS):
            m[h * HID:(h + 1) * HID, h] = a[h]
        return m

    def wext(W, al, ar, row_perm):
        """[256, 264] = [W(cols interleaved) | W@Al | W@Ar], rows optionally
        permuted (for layer 2 whose input features are interleaved)."""
        W = np.asarray(W, np.float64)
        m = np.concatenate([W[:, _PERM], W @ head_mat(al), W @ head_mat(ar)], axis=1)
        if row_perm is not None:
            m = m[row_perm]
        out = np.zeros((P, 2 * 264), np.float16)
        for g in range(2):
            out[:, g * 264:(g + 1) * 264] = m[g * P:(g + 1) * P].astype(np.float16)
        return out

    W1k = wext(W1, al1, ar1, None)
    W2k = wext(W2, al2, ar2, _PERM)
    iota = np.tile(np.arange(P, dtype=np.float16), (P, 1))
    ident = np.eye(P, dtype=np.float16)

    in_maps = []
    for c in range(N_CORES):
        xl = np.zeros((NPAD, IN_DIM), np.float32)
        mine = core_of_n == c
        xl[slot_of_n[mine]] = x[mine]
        xT = np.zeros((P, 2 * NPAD), np.float16)
        for g in range(2):
            xT[:, g * NPAD:(g + 1) * NPAD] = xl[:, g * P:(g + 1) * P].T.astype(np.float16)

        def build_phase(elists, rows, T):
            idx_cols = []
            ldst_cols = np.full((P, max(sum(T), 1)), PAD_LDST, np.float32)
            toff = 0
            for b in range(NB):
                el = elists[c][b]
                n = T[b] * P
                if n == 0:
                    continue
                ii = [int(rows[e]) for e in el] + [0] * (n - len(el))
                lv = ([float(lin_all[e]) for e in el]
                      + [PAD_LDST] * (n - len(el)))
                idx_cols.append(_wrap_idx(ii))
                ldst_cols[:, toff:toff + T[b]] = \
                    np.asarray(lv, np.float32).reshape(T[b], P).T
                toff += T[b]
            idx = (np.concatenate(idx_cols, axis=1) if idx_cols
                   else np.zeros((P, 8), np.int16))
            return idx, ldst_cols

        idxA, ldstA = build_phase(eA, rowA, T_A)
        idxB, ldstB = build_phase(eB, rowB, T_B)

        in_maps.append({
            "xT": xT, "W1k": W1k, "W2k": W2k,
            "idxA": idxA, "idxB": idxB,
            "ldstA": ldstA, "ldstB": ldstB,
            "iota": iota, "ident": ident,
        })

    plan["idxA_cols"] = in_maps[0]["idxA"].shape[1]
    plan["idxB_cols"] = in_maps[0]["idxB"].shape[1]
    plan["ldstA_cols"] = in_maps[0]["ldstA"].shape[1]
    plan["ldstB_cols"] = in_maps[0]["ldstB"].shape[1]
    return in_maps, plan


def _build(plan):
    import os
    KSIM = int(os.environ.get("KSIM", "0"))
    KAG = int(os.environ.get("KAG", "1"))
    KAGPOS = int(os.environ.get("KAGPOS", "2"))
    KEDGE = int(os.environ.get("KEDGE", "1"))
    KGOFF = int(os.environ.get("KGOFF", "0"))
    import concourse.bass as bass
    import concourse.bacc as bacc
    import concourse.mybir as mybir
    import concourse.tile as tile

    dt = mybir.dt
    F16 = dt.float16
    T_A, T_B = plan["T_A"], plan["T_B"]

    nc = bacc.Bacc("TRN2", target_bir_lowering=False, debug=False,
                   num_devices=(1 if KSIM else N_CORES),
                   num_swdge_queues=4)
    xT_ap = nc.dram_tensor("xT", [P, 2 * NPAD], F16, kind="ExternalInput").ap()
    W1k_ap = nc.dram_tensor("W1k", [P, 2 * 264], F16, kind="ExternalInput").ap()
    W2k_ap = nc.dram_tensor("W2k", [P, 2 * 264], F16, kind="ExternalInput").ap()
    idxA_ap = nc.dram_tensor("idxA", [P, plan["idxA_cols"]], dt.int16, kind="ExternalInput").ap()
    idxB_ap = nc.dram_tensor("idxB", [P, plan["idxB_cols"]], dt.int16, kind="ExternalInput").ap()
    ldstA_ap = nc.dram_tensor("ldstA", [P, plan["ldstA_cols"]], dt.float32, kind="ExternalInput").ap()
    ldstB_ap = nc.dram_tensor("ldstB", [P, plan["ldstB_cols"]], dt.float32, kind="ExternalInput").ap()
    iota_ap = nc.dram_tensor("iota", [P, P], F16, kind="ExternalInput").ap()
    ident_ap = nc.dram_tensor("ident", [P, P], F16, kind="ExternalInput").ap()
    out_ap = nc.dram_tensor("out", [NPAD, IN_DIM], dt.float32, kind="ExternalOutput").ap()

    AF = mybir.ActivationFunctionType
    ALU = mybir.AluOpType

    with tile.TileContext(nc) as tc:
        with tc.tile_pool(name="const", bufs=1) as cpool, \
             tc.tile_pool(name="gemm", bufs=2) as gpool, \
             tc.tile_pool(name="edge", bufs=2) as epool, \
             tc.tile_pool(name="flush", bufs=2) as fpool, \
             tc.tile_pool(name="psum", bufs=2, space="PSUM") as pp, \
             tc.tile_pool(name="dram", bufs=1, space="DRAM") as dram:

            iota_t = cpool.tile([P, P], F16)
            ident_t = cpool.tile([P, P], F16)
            idxA_t = cpool.tile([P, plan["idxA_cols"]], dt.int16)
            idxB_t = cpool.tile([P, plan["idxB_cols"]], dt.int16)
            ldstA_t = cpool.tile([P, plan["ldstA_cols"]], dt.float32)
            ldstB_t = cpool.tile([P, plan["ldstB_cols"]], dt.float32)
            w1_t = cpool.tile([P, 2 * 264], F16)
            w2_t = cpool.tile([P, 2 * 264], F16)
            nc.sync.dma_start(iota_t[:], iota_ap[:])
            nc.sync.dma_start(ident_t[:], ident_ap[:])
            nc.sync.dma_start(idxA_t[:], idxA_ap[:])
            nc.sync.dma_start(idxB_t[:], idxB_ap[:])
            nc.sync.dma_start(ldstA_t[:], ldstA_ap[:])
            nc.sync.dma_start(ldstB_t[:], ldstB_ap[:])
            nc.sync.dma_start(w1_t[:], W1k_ap[:])
            nc.sync.dma_start(w2_t[:], W2k_ap[:])

            # per-layer er values [dst-lane, 4], SBUF resident
            er_all = [cpool.tile([P, NB * HEADS], F16, name=f"er_all{i}")
                      for i in range(2)]
            # phase-A partial aggregation stash
            stash = cpool.tile([P, NB * CG], F16)

            tabA_loc = dram.tile([A_ROWS, ESL], F16)
            tabB_loc = dram.tile([B_ROWS, ESL], F16)
            _ashared = "Local" if KSIM else "Shared"
            tabA1 = dram.tile([RA, ESL], F16, addr_space=_ashared)
            tabB1 = dram.tile([RB, ESL], F16, addr_space=_ashared)
            tabA2 = dram.tile([RA, ESL], F16, addr_space=_ashared)
            tabB2 = dram.tile([RB, ESL], F16, addr_space=_ashared)
            h1T = dram.tile([P, 2 * NPAD], F16)

            def gemm_block(layer, b):
                wk = w1_t if layer == 1 else w2_t
                ps = pp.tile([P, 264], dt.float32, space="PSUM", name="gemm_ps", tag="gemm_ps")
                for g in range(2):
                    hk = gpool.tile([P, P], F16, name="hk", tag="hk")
                    if layer == 1:
                        nc.sync.dma_start(hk[:], xT_ap[:, g * NPAD + b * P: g * NPAD + (b + 1) * P])
                    else:
                        nc.sync.dma_start(hk[:], h1T[:, g * NPAD + b * P: g * NPAD + (b + 1) * P])
                    nc.tensor.matmul(out=ps[:], lhsT=hk[:], rhs=wk[:, g * 264:(g + 1) * 264],
                                     start=(g == 0), stop=(g == 1))
                sb = gpool.tile([P, ESL], F16, name="gemm_sb", tag="gemm_sb")
                nc.scalar.activation(sb[:, 0:CG], ps[:, 0:CG], AF.Copy)
                nc.vector.memset(sb[:, CG:ESL], 0.0)
                nc.vector.tensor_copy(er_all[layer - 1][:, b * 4:(b + 1) * 4], ps[:, 260:264])
                tab_loc = tabA_loc if b < NBA else tabB_loc
                r0 = b * P if b < NBA else (b - NBA) * P
                nc.sync.dma_start(tab_loc[r0:r0 + P, :], sb[:])

            def edge_phase(layer, phase):
                T = T_A if phase == 0 else T_B
                idx_t = idxA_t if phase == 0 else idxB_t
                ldst_t = ldstA_t if phase == 0 else ldstB_t
                if phase == 0:
                    tab = tabA1 if layer == 1 else tabA2
                else:
                    tab = tabB1 if layer == 1 else tabB2
                erl = er_all[layer - 1]

                # quad-merged gathers
                toff = 0  # tile offset within this phase
                for q0 in range(0, NB, QUAD):
                    blocks = [b for b in range(q0, min(q0 + QUAD, NB)) if T[b] > 0]
                    tq = sum(T[b] for b in blocks)
                    if tq == 0:
                        for b in range(q0, min(q0 + QUAD, NB)):
                            finish_block(layer, phase, b, None, None)
                        continue
                    if KGOFF:
                        G = None
                    else:
                        G = epool.tile([P, tq * ESL], F16, name="G", tag="G")
                        nc.gpsimd.dma_gather(
                            out_ap=G[:].rearrange("p (t e) -> p t e", e=ESL),
                            in_ap=tab[:], idxs_ap=idx_t[:, 8 * toff: 8 * (toff + tq)],
                            num_idxs=tq * P, num_idxs_reg=tq * P, elem_size=ESL,
                            single_packet=False, queue_num=(q0 // QUAD) % 4)
                    goff = 0  # tile offset within G
                    for b in range(q0, min(q0 + QUAD, NB)):
                        t = T[b]
                        if t == 0 or not KEDGE or G is None:
                            finish_block(layer, phase, b, None, None)
                            continue
                        process_block(layer, phase, b, t, G, goff, ldst_t, toff + goff, erl)
                        goff += t
                    toff += tq

            def process_block(layer, phase, b, t, G, goff, ldst_t, loff, erl):
                # selection matrix MT[e, d] = (d == ldst[e])
                MT = epool.tile([P, t * P], F16, name="MT", tag="MT")
                for ti in range(t):
                    nc.vector.tensor_scalar(
                        out=MT[:, ti * P:(ti + 1) * P], in0=iota_t[:],
                        scalar1=ldst_t[:, loff + ti: loff + ti + 1], scalar2=None,
                        op0=ALU.is_equal)
                # MT2 = MT^T per tile (PE transpose, 8 tiles per 2KB PSUM bank)
                MT2 = epool.tile([P, t * P], F16, name="MT2", tag="MT2")
                for t0 in range(0, t, 8):
                    n8 = min(8, t - t0)
                    trp = pp.tile([P, 8 * P], F16, space="PSUM", name="tr_ps", tag="tr_ps")
                    for k in range(n8):
                        nc.tensor.transpose(out=trp[:, k * P:(k + 1) * P],
                                            in_=MT[:, (t0 + k) * P:(t0 + k + 1) * P],
                                            identity=ident_t[:])
                    nc.scalar.activation(MT2[:, t0 * P:(t0 + n8) * P],
                                         trp[:, 0:n8 * P], AF.Copy)
                # er per edge: ER[e, h] = sum_c MT2[c, e] * er[c, h]
                er_ps = pp.tile([P, t * 4], dt.float32, space="PSUM", name="er_ps", tag="er_ps", bufs=1)
                for ti in range(t):
                    nc.tensor.matmul(out=er_ps[:, ti * 4:(ti + 1) * 4],
                                     lhsT=MT2[:, ti * P:(ti + 1) * P],
                                     rhs=erl[:, b * 4:(b + 1) * 4],
                                     start=True, stop=True, skip_group_check=True)
                # e-chain
                g3 = G[:, goff * ESL:(goff + t) * ESL].rearrange("p (t c) -> p t c", c=ESL)
                E = epool.tile([P, t * 4], dt.float32, name="E", tag="E")
                e3 = E[:].rearrange("p (t h) -> p t h", h=4)
                nc.vector.tensor_tensor(out=e3, in0=g3[:, :, 256:260],
                                        in1=er_ps[:].rearrange("p (t h) -> p t h", h=4),
                                        op=ALU.add)
                L = epool.tile([P, t * 4], dt.float32, name="L", tag="L")
                nc.vector.tensor_scalar_mul(L[:], E[:], NEG_SLOPE)
                nc.vector.tensor_tensor(out=L[:], in0=E[:], in1=L[:], op=ALU.max)
                # W = [feat * X | X], X written straight into cols 256:260 by ACT
                W = epool.tile([P, t * CG], F16, name="W", tag="W")
                w3 = W[:].rearrange("p (t c) -> p t c", c=CG)
                nc.scalar.activation(w3[:, :, 256:260],
                                     L[:].rearrange("p (t h) -> p t h", h=4), AF.Exp)
                w4 = w3[:, :, 0:256].rearrange("p t (j h) -> p t j h", h=4)
                gf4 = g3[:, :, 0:256].rearrange("p t (j h) -> p t j h", h=4)
                x4 = w3[:, :, 256:260].rearrange("p t (o h) -> p t o h", o=1) \
                                      .to_broadcast([P, t, 64, 4])
                nc.vector.tensor_tensor(out=w4, in0=gf4, in1=x4, op=ALU.mult)
                # aggregate
                agg = pp.tile([P, CG], dt.float32, space="PSUM", name="agg_ps", tag="agg_ps")
                for ti in range(t):
                    nc.tensor.matmul(out=agg[:], lhsT=MT[:, ti * P:(ti + 1) * P],
                                     rhs=W[:, ti * CG:(ti + 1) * CG],
                                     start=(ti == 0), stop=(ti == t - 1))
                finish_block(layer, phase, b, agg, None)

            def finish_block(layer, phase, b, agg, _unused):
                if phase == 0:
                    # stash phase-A partials (or zeros if no A edges)
                    if agg is None:
                        nc.vector.memset(stash[:, b * CG:(b + 1) * CG], 0.0)
                    else:
                        nc.scalar.activation(stash[:, b * CG:(b + 1) * CG],
                                             agg[:], AF.Copy)
                    return
                # phase B: combine + softmax divide + ELU
                comb = fpool.tile([P, CG], dt.float32, name="comb", tag="comb")
                if agg is None:
                    nc.vector.tensor_copy(comb[:], stash[:, b * CG:(b + 1) * CG])
                else:
                    nc.vector.tensor_tensor(out=comb[:], in0=stash[:, b * CG:(b + 1) * CG],
                                            in1=agg[:], op=ALU.add)
                dmx = fpool.tile([P, 4], dt.float32, name="dmx", tag="dmx")
                nc.vector.tensor_scalar_max(dmx[:], comb[:, 256:260], 1e-30)
                rec = fpool.tile([P, 4], dt.float32, name="rec", tag="rec")
                nc.vector.reciprocal(rec[:], dmx[:])
                ob = fpool.tile([P, 256], dt.float32, name="ob", tag="ob")
                ob4 = ob[:].rearrange("p (j h) -> p j h", h=4)
                rec4 = rec[:].rearrange("p (o h) -> p o h", o=1).to_broadcast([P, 64, 4])
                nc.vector.tensor_tensor(out=ob4,
                                        in0=comb[:, 0:256].rearrange("p (j h) -> p j h", h=4),
                                        in1=rec4, op=ALU.mult)
                # ELU: relu(x) + exp(min(x,0)) - 1
                nb_t = fpool.tile([P, 256], dt.float32, name="nb", tag="nb")
                nc.vector.tensor_scalar_min(nb_t[:], ob[:], 0.0)
                en = fpool.tile([P, 256], dt.float32, name="en", tag="en")
                nc.scalar.activation(en[:], nb_t[:], AF.Exp)
                pb = fpool.tile([P, 256], dt.float32, name="pb", tag="pb")
                nc.scalar.activation(pb[:], ob[:], AF.Relu)
                if layer == 1:
                    fb = fpool.tile([P, 256], F16, name="fb", tag="fb")
                    nc.vector.tensor_tensor(out=fb[:], in0=en[:], in1=pb[:], op=ALU.add)
                    nc.vector.tensor_scalar_add(fb[:], fb[:], -1.0)
                    for g in range(2):
                        tsb = fpool.tile([P, P], F16, name="tsb", tag="tsb")
                        nc.sync.dma_start(tsb[:], fb[:, g * P:(g + 1) * P],
                                          transpose=True)
                        nc.sync.dma_start(h1T[:, g * NPAD + b * P: g * NPAD + (b + 1) * P], tsb[:])
                    # layer-2 GEMM for this block as soon as its h1 lands
                    gemm_block(2, b)
                    if KAGPOS:
                        agb = NBA - 1 if KAGPOS == 1 else (NB - 2 if KAGPOS == 2 else NB - 1)
                        if b == agb:
                            ag(tabA_loc, tabA2)
                        elif b == NB - 1:
                            ag(tabB_loc, tabB2)
                else:
                    fb = fpool.tile([P, 256], dt.float32, name="fb32", tag="fb32")
                    nc.vector.tensor_tensor(out=fb[:], in0=en[:], in1=pb[:], op=ALU.add)
                    nc.vector.tensor_scalar_add(fb[:], fb[:], -1.0)
                    nc.sync.dma_start(out_ap[b * P:(b + 1) * P, :], fb[:])

            def ag(src_tile, dst_tile):
                if KSIM or not KAG:
                    return
                nc.gpsimd.collective_compute(
                    "AllGather", mybir.AluOpType.bypass,
                    replica_groups=[list(range(N_CORES))],
                    ins=[src_tile.opt()],
                    outs=[dst_tile.opt()])

            # ---- schedule ----
            for b in range(NBA):
                gemm_block(1, b)
            ag(tabA_loc, tabA1)
            for b in range(NBA, NB):
                gemm_block(1, b)
            ag(tabB_loc, tabB1)
            edge_phase(1, 0)
            edge_phase(1, 1)   # interleaves gemm_block(2, b) + AG2 launches
            if not KAGPOS:
                ag(tabA_loc, tabA2)
                ag(tabB_loc, tabB2)
            edge_phase(2, 0)
            edge_phase(2, 1)

    nc.compile()
    return nc


def _finish(results):
    """Per-core 'out' [NPAD, 256] (cols interleaved j*4+h, rows placed by
    _placement) -> tuple of heads."""
    outs = np.stack([np.asarray(results[c]["out"]) for c in range(N_CORES)])
    h = outs[_PLACE["core"], _PLACE["slot"]]                # [N, 256]
    h = h.reshape(N_NODES, HID, HEADS).transpose(0, 2, 1)   # [N, H, D]
    return tuple(h[:, i] for i in range(HEADS))


def kernel(**inputs):
    from concourse.bass_utils import run_bass_kernel_spmd
    in_maps, plan = _prep(inputs["x"], inputs["src"], inputs["dst"],
                          inputs["W1"], inputs["al1"], inputs["ar1"],
                          inputs["W2"], inputs["al2"], inputs["ar2"])
    nc = _build(plan)
    res = run_bass_kernel_spmd(nc, in_maps, core_ids=list(range(N_CORES)),
                               trace=False)
    return _finish(res.results)
